# revision 23
# baseline (speedup 1.0000x reference)
"""TRN2 8-core SPMD kernel for nn_DecoderBlock_13443247636967.

Math note (validated to rel err ~1.5e-7 against the fp32 reference):
the reference uses SCALE = head_size**-5 = 2**-30 ~ 9.3e-10, so every
pre-softmax score satisfies |s| < 4e-8.  exp(s - max) is then 1.0 to
within one fp32 ulp and the reference softmax IS the uniform causal
average w_u = 1/(t+1) at fp32 precision.  Attention therefore reduces
to a causal prefix-mean of V, and the per-head structure fuses into a
single [D, D] value projection (Wk enters only through the vanishing
scores, so it cannot affect the output at fp32 resolution).

Sharding: core c = (batch b = c//2, half = c%2) owns 1024 sequence rows
of one batch.  The only cross-row coupling is the prefix sum.  Within a
core the 8 row-tiles form a short serial chain: the prefix matmul
C_ps = triu^T @ V already computes the full tile column-sum in its last
row, so row 127 of the (carry-added, unscaled) prefix PSUM *is* the
next tile's carry.  The carry is re-injected into PSUM with an all-ones
stationary matmul against a [P, D] tile whose partitions 1..127 are
zero.  The chain's root (sum over the other core's 1024 rows, pushed
through Wv) is a single [1, D] vector computed on the host during
input prep.  No collectives.

Precision: matmuls run in bf16 (keeps the PE's fast-weight-load path,
which f32r disables); residuals and LayerNorm stats stay fp32.
Measured end-to-end relative error vs the fp32 reference: ~1e-3-ish,
tolerance is 2e-2.
"""

import numpy as np
import ml_dtypes

import concourse.bass as bass
import concourse.mybir as mybir
import concourse.tile as tile
from concourse import bacc
from concourse.bass_utils import run_bass_kernel_spmd
from concourse.masks import make_identity

P = 128          # partitions / row-tile height
D = 1024         # model dim
TH = 1024        # sequence rows per core
NT = TH // P     # 8 row tiles
KC = D // P      # 8 contraction chunks
NF = 512         # matmul max moving free dim
NH = D // NF     # 2 column halves
B, T = 4, 2048
EPS = 1e-5
F32 = mybir.dt.float32
BF16 = mybir.dt.bfloat16


def _build(lean=True):
    # lean: biases known-zero and LN gains known-one (checked host-side;
    # the general variant is compiled on demand if that ever fails)
    nc = bacc.Bacc(
        "TRN2", target_bir_lowering=False, debug=False, num_devices=8
    )
    x = nc.dram_tensor("x_half", [TH, D], F32, kind="ExternalInput").ap()
    xT = nc.dram_tensor("xT_half", [NT, P, KC, P], BF16, kind="ExternalInput").ap()
    Wv = nc.dram_tensor("Wv", [D, D], BF16, kind="ExternalInput").ap()
    Wo = nc.dram_tensor("Wo", [D, D], BF16, kind="ExternalInput").ap()
    Wf1 = nc.dram_tensor("Wf1", [D, D], BF16, kind="ExternalInput").ap()
    Wf2 = nc.dram_tensor("Wf2", [D, D], BF16, kind="ExternalInput").ap()
    vecs = {
        name: nc.dram_tensor(name, [1, D], F32, kind="ExternalInput").ap()
        for name in ["bo", "bf1", "bf2", "g1", "b1", "g2", "b2"]
    }
    invcnt = nc.dram_tensor("invcnt", [P, NT], F32, kind="ExternalInput").ap()
    ut_in = nc.dram_tensor("ut_b", [P, P], BF16, kind="ExternalInput").ap()
    # carry0_t: zeros except row 127 = colsum(x_prev) @ Wv (host-computed)
    carry0 = nc.dram_tensor("carry0_t", [P, D], BF16, kind="ExternalInput").ap()
    # cnt_rows[127, j, :] = multiplier turning C_prev row 127 into carry_j
    cnt_in = nc.dram_tensor("cnt_rows", [P, NT, P], BF16, kind="ExternalInput").ap()
    out = nc.dram_tensor("out", [TH, D], F32, kind="ExternalOutput").ap()

    with tile.TileContext(nc) as tc:
        with tc.tile_pool(name="w", bufs=4) as wpool, \
             tc.tile_pool(name="xs", bufs=4) as xpool, \
             tc.tile_pool(name="bc", bufs=4) as bcpool, \
             tc.tile_pool(name="wk", bufs=16) as wkpool, \
             tc.tile_pool(name="tp", bufs=4) as tppool, \
             tc.tile_pool(name="rows", bufs=1) as rows, \
             tc.tile_pool(name="stat", bufs=4) as statpool, \
             tc.tile_pool(name="pmm", bufs=6, space="PSUM") as pmm, \
             tc.tile_pool(name="ptp", bufs=2, space="PSUM") as ptp:

            # ---- constants ----
            ident = rows.tile([P, P], BF16)
            make_identity(nc, ident)
            ut_b = rows.tile([P, P], BF16)
            nc.sync.dma_start(out=ut_b, in_=ut_in)
            eps_t = rows.tile([P, 1], F32)
            nc.vector.memset(eps_t, EPS)
            icnt = rows.tile([P, NT], F32)
            nc.sync.dma_start(out=icnt, in_=invcnt)
            carry0_sb = rows.tile([P, D], BF16)
            nc.sync.dma_start(out=carry0_sb, in_=carry0)
            cnt_rows = rows.tile([P, NT, P], BF16)
            nc.sync.dma_start(out=cnt_rows, in_=cnt_in)

            def load_w(ap, name):
                w = wpool.tile([P, KC, D], BF16, tag="W", name=name)
                resh = ap.rearrange("(kc p) n -> p kc n", p=P)
                # one dma_start per kc chunk -> 8 rings in parallel
                for kc in range(KC):
                    nc.sync.dma_start(
                        out=w[:, kc:kc + 1, :], in_=resh[:, kc:kc + 1, :]
                    )
                return w

            def load_bc(name):
                t = bcpool.tile([P, D], F32, tag="bc", name=f"bc_{name}")
                nc.sync.dma_start(out=t, in_=vecs[name].to_broadcast([P, D]))
                return t

            def transpose_blocks(src, name):
                """src [P, D] bf16 natural -> [P, KC, P] bf16 blocks^T."""
                dst = tppool.tile([P, KC, P], BF16, tag="tp", name=name)
                for g in range(2):
                    tp_ps = ptp.tile([P, 4 * P], BF16, tag="ptp")
                    for k4 in range(4):
                        kc = g * 4 + k4
                        nc.tensor.transpose(
                            tp_ps[:, k4 * P:(k4 + 1) * P],
                            src[:, kc * P:(kc + 1) * P],
                            ident,
                        )
                    nc.vector.tensor_copy(
                        out=dst[:, g * 4:(g + 1) * 4, :],
                        in_=tp_ps.rearrange("p (k q) -> p k q", k=4),
                    )
                return dst

            def mm_group(lhsT_blocks, w_sb, n):
                """psum = sum_kc lhsT[:,kc,:].T @ w[:,kc,n-half]"""
                ps = pmm.tile([P, NF], F32, tag="mm")
                nsl = slice(n * NF, (n + 1) * NF)
                for kc in range(KC):
                    nc.tensor.matmul(
                        ps,
                        lhsT=lhsT_blocks[:, kc, :],
                        rhs=w_sb[:, kc, nsl],
                        start=(kc == 0),
                        stop=(kc == KC - 1),
                    )
                return ps

            def layernorm(src, dst, g_bc, b_bc):
                st = statpool.tile([P, NH, 6], F32, tag="st")
                for h in range(NH):
                    nc.vector.bn_stats(
                        out=st[:, h, :], in_=src[:, h * NF:(h + 1) * NF]
                    )
                mv = statpool.tile([P, 2], F32, tag="mv")
                nc.vector.bn_aggr(out=mv, in_=st)
                rstd = statpool.tile([P, 1], F32, tag="rs")
                nc.scalar.activation(
                    out=rstd,
                    in_=mv[:, 1:2],
                    func=mybir.ActivationFunctionType.Sqrt,
                    bias=eps_t,
                    scale=1.0,
                )
                nc.vector.reciprocal(out=rstd, in_=rstd)
                # dst = src*rstd - mean*rstd on ACT, then g/b on DVE
                mb = statpool.tile([P, 1], F32, tag="mb")
                nc.vector.tensor_scalar(
                    out=mb, in0=mv[:, 0:1], scalar1=rstd, scalar2=-1.0,
                    op0=mybir.AluOpType.mult, op1=mybir.AluOpType.mult,
                )
                # normalize: half 0 on DVE, half 1 on ACT (parallel)
                nc.vector.tensor_scalar(
                    out=dst[:, 0:NF], in0=src[:, 0:NF],
                    scalar1=rstd, scalar2=mb,
                    op0=mybir.AluOpType.mult, op1=mybir.AluOpType.add,
                )
                nc.scalar.activation(
                    out=dst[:, NF:D], in_=src[:, NF:D],
                    func=mybir.ActivationFunctionType.Identity,
                    bias=mb, scale=rstd,
                )
                if not lean:
                    nc.vector.tensor_mul(out=dst, in0=dst, in1=g_bc)
                    nc.vector.tensor_add(out=dst, in0=dst, in1=b_bc)

            # ==== software-pipelined per-tile loop ====
            # attention of tile j is interleaved with the FFN of tile j-2
            # so every cross-engine wait on one stage is covered by
            # independent PE work from the other.
            state = {"C_prev": carry0_sb}
            xT_t = [None] * NT
            x_t = [None] * NT

            def fetch(j):
                if j >= NT or xT_t[j] is not None:
                    return
                xT_t[j] = tppool.tile([P, KC, P], BF16, tag="xT", name="xT")
                x_t[j] = xpool.tile([P, D], F32, tag="x", name="x1")
                for h in (0, 1):
                    ksl = slice(h * (KC // 2), (h + 1) * (KC // 2))
                    nc.sync.dma_start(out=xT_t[j][:, ksl, :], in_=xT[j][:, ksl, :])
                    nsl = slice(h * NF, (h + 1) * NF)
                    nc.sync.dma_start(
                        out=x_t[j][:, nsl], in_=x[j * P:(j + 1) * P, nsl]
                    )

            # fetch the first tiles BEFORE the weight DMAs so the first V
            # matmul isn't queued behind 8 MB of weight traffic
            fetch(0)
            fetch(1)
            Wv_sb = load_w(Wv, "Wv")
            Wo_sb = load_w(Wo, "Wo")
            Wf1_sb = load_w(Wf1, "Wf1")
            Wf2_sb = load_w(Wf2, "Wf2")
            bo_bc = None if lean else load_bc("bo")
            g1_bc = None if lean else load_bc("g1")
            b1_bc = None if lean else load_bc("b1")
            bf1_bc = None if lean else load_bc("bf1")
            bf2_bc = None if lean else load_bc("bf2")
            g2_bc = None if lean else load_bc("g2")
            b2_bc = None if lean else load_bc("b2")

            def copy_halves(dst, srcs, scalars=None, relu=False):
                """half 0 on DVE, half 1 on ACT (parallel engines)."""
                for n in range(NH):
                    nsl = slice(n * NF, (n + 1) * NF)
                    sc = scalars[n] if scalars is not None else 1.0
                    if n == 0:
                        if relu:
                            nc.vector.tensor_scalar_max(
                                out=dst[:, nsl], in0=srcs[n], scalar1=0.0
                            )
                        elif scalars is not None:
                            nc.vector.tensor_scalar_mul(
                                out=dst[:, nsl], in0=srcs[n], scalar1=sc
                            )
                        else:
                            nc.vector.tensor_copy(
                                out=dst[:, nsl], in_=srcs[n]
                            )
                    else:
                        fn = (mybir.ActivationFunctionType.Relu if relu
                              else mybir.ActivationFunctionType.Identity)
                        nc.scalar.activation(
                            out=dst[:, nsl], in_=srcs[n], func=fn,
                            scale=sc if scalars is not None else 1.0,
                        )

            def attn_v(j):
                """V matmuls + staging copies."""
                V_sb = wkpool.tile([P, D], BF16, tag="wk", name="V")
                V_ps = [mm_group(xT_t[j], Wv_sb, n) for n in range(NH)]
                copy_halves(V_sb, V_ps)
                fetch(j + 1)
                return V_sb

            def attn_prefix(j, V_sb):
                """C = (triu^T @ V + carry_j) * invcnt; carry_j comes from
                row 127 of the previous scaled C via a one-hot-row
                stationary operand holding the count."""
                C_b = wkpool.tile([P, D], BF16, tag="wk", name="C")
                pss = []
                for n in range(NH):
                    nsl = slice(n * NF, (n + 1) * NF)
                    ps = pmm.tile([P, NF], F32, tag="mm")
                    nc.tensor.matmul(
                        ps, lhsT=ut_b, rhs=V_sb[:, nsl],
                        start=True, stop=False,
                    )
                    nc.tensor.matmul(
                        ps, lhsT=cnt_rows[:, j, :],
                        rhs=state["C_prev"][:, nsl],
                        start=False, stop=True,
                    )
                    pss.append(ps)
                copy_halves(
                    C_b, pss, scalars=[icnt[:, j:j + 1]] * NH
                )
                state["C_prev"] = C_b
                return C_b

            def attn_out(j, CT):
                """AO = C @ Wo ; r1 = AO + x ; N1 = LN1(r1)."""
                r1 = wkpool.tile([P, D], F32, tag="wk", name="r1")
                for n in range(NH):
                    nsl = slice(n * NF, (n + 1) * NF)
                    ps = mm_group(CT, Wo_sb, n)
                    if lean:
                        nc.vector.tensor_add(
                            out=r1[:, nsl], in0=ps, in1=x_t[j][:, nsl]
                        )
                    else:
                        nc.vector.tensor_add(
                            out=r1[:, nsl], in0=ps, in1=bo_bc[:, nsl]
                        )
                if not lean:
                    nc.vector.tensor_add(out=r1, in0=r1, in1=x_t[j])
                N1_b = wkpool.tile([P, D], BF16, tag="wk", name="N1")
                layernorm(r1, N1_b, g1_bc, b1_bc)
                return N1_b

            def ffn_a(j, N1_b):
                """H = relu(N1 @ Wf1)."""
                N1T = transpose_blocks(N1_b, "N1T")
                H_b = wkpool.tile([P, D], BF16, tag="wk", name="H")
                H_ps = [mm_group(N1T, Wf1_sb, n) for n in range(NH)]
                if lean:
                    copy_halves(H_b, H_ps, relu=True)
                else:
                    for n in range(NH):
                        nsl = slice(n * NF, (n + 1) * NF)
                        nc.vector.tensor_add(
                            out=H_b[:, nsl], in0=H_ps[n], in1=bf1_bc[:, nsl]
                        )
                    nc.vector.tensor_scalar_max(out=H_b, in0=H_b, scalar1=0.0)
                return H_b

            def ffn_b(j, N1_b, H_b, last=False):
                """z = H @ Wf2 + N1 + x ; out = LN2(z)."""
                HT = transpose_blocks(H_b, "HT")
                z = wkpool.tile([P, D], F32, tag="wk", name="z")
                for n in range(NH):
                    nsl = slice(n * NF, (n + 1) * NF)
                    ps = mm_group(HT, Wf2_sb, n)
                    if lean:
                        nc.vector.tensor_add(
                            out=z[:, nsl], in0=ps, in1=N1_b[:, nsl]
                        )
                    else:
                        nc.vector.tensor_add(
                            out=z[:, nsl], in0=ps, in1=bf2_bc[:, nsl]
                        )
                if not lean:
                    nc.vector.tensor_add(out=z, in0=z, in1=N1_b)
                if last:
                    # tail: DVE beats GpSimd's fixed overhead
                    nc.vector.tensor_add(out=z, in0=z, in1=x_t[j])
                else:
                    nc.gpsimd.tensor_add(out=z, in0=z, in1=x_t[j])
                o = wkpool.tile([P, D], F32, tag="wk", name="o")
                layernorm(z, o, g2_bc, b2_bc)
                # 4-way split -> 4 rings; matters for the last tile's tail
                for q in range(4):
                    qsl = slice(q * (D // 4), (q + 1) * (D // 4))
                    nc.sync.dma_start(
                        out=out[j * P:(j + 1) * P, qsl], in_=o[:, qsl]
                    )

            stage = {}  # j -> (N1_b, H_b)
            for j in range(NT):
                V_sb = attn_v(j)
                if j >= 2:
                    stage[j - 2] = (stage[j - 2], ffn_a(j - 2, stage[j - 2]))
                C_b = attn_prefix(j, V_sb)
                # CT transposes + copies BEFORE ffn_b's DVE work so Wo
                # doesn't queue behind the previous tile's z/LN2 chain
                CT = transpose_blocks(C_b, "CT")
                if j >= 2:
                    ffn_b(j - 2, *stage.pop(j - 2))
                N1_b = attn_out(j, CT)
                stage[j] = N1_b
            # epilogue: interleave the two trailing FFNs
            H_a = ffn_a(NT - 2, stage[NT - 2])
            H_b_ = ffn_a(NT - 1, stage[NT - 1])
            ffn_b(NT - 2, stage.pop(NT - 2), H_a)
            ffn_b(NT - 1, stage.pop(NT - 1), H_b_, last=True)

    nc.compile()
    return nc


_CACHE = {}


def _get_nc(lean=True):
    key = "lean" if lean else "general"
    if key not in _CACHE:
        _CACHE[key] = _build(lean=lean)
    return _CACHE[key]


def _bf16(a):
    return np.ascontiguousarray(np.asarray(a, np.float32)).astype(
        ml_dtypes.bfloat16
    )


def _in_maps(x, Wv, Wo, bo, g1, b1, Wf1, bf1, Wf2, bf2, g2, b2):
    x = np.asarray(x, dtype=np.float32)
    Wv_all = np.ascontiguousarray(
        np.asarray(Wv, np.float32).transpose(1, 0, 2).reshape(D, D)
    )
    base = {
        "Wv": _bf16(Wv_all),
        "Wo": _bf16(Wo),
        "Wf1": _bf16(Wf1),
        "Wf2": _bf16(Wf2),
        "bo": np.asarray(bo, np.float32).reshape(1, D),
        "bf1": np.asarray(bf1, np.float32).reshape(1, D),
        "bf2": np.asarray(bf2, np.float32).reshape(1, D),
        "g1": np.asarray(g1, np.float32).reshape(1, D),
        "b1": np.asarray(b1, np.float32).reshape(1, D),
        "g2": np.asarray(g2, np.float32).reshape(1, D),
        "b2": np.asarray(b2, np.float32).reshape(1, D),
        "ut_b": _bf16(np.triu(np.ones((P, P), np.float32))),
    }
    in_maps = []
    for c in range(8):
        b, half = divmod(c, 2)
        t0 = half * TH
        icnt = 1.0 / (
            t0 + np.arange(P)[:, None] + P * np.arange(NT)[None, :] + 1.0
        )
        m = dict(base)
        xh = np.ascontiguousarray(x[b, t0:t0 + TH])
        m["x_half"] = xh
        # [NT, P, KC, P]: per row-tile j, partition p holds the KC
        # contraction blocks of x^T contiguously (2KB DMA lines)
        xt = xh.T.reshape(KC, P, NT, P).transpose(2, 1, 0, 3)
        m["xT_half"] = _bf16(np.ascontiguousarray(xt))
        m["invcnt"] = icnt.astype(np.float32)
        # prefix-sum root: column-sums of the other core's rows through Wv,
        # staged in row 127 of an otherwise-zero [P, D] tile
        c0 = np.zeros((P, D), np.float32)
        if half:
            c0[P - 1] = x[b, 0:TH].sum(axis=0) @ Wv_all
        m["carry0_t"] = _bf16(c0)
        # cnt_rows[127, 0] = 1 (consumes carry0 as-is); for j>=1 the
        # multiplier cnt = t0 + 128*j undoes invcnt on C_prev's row 127
        cr = np.zeros((P, NT, P), np.float32)
        cr[P - 1, 0, :] = 1.0
        for j in range(1, NT):
            cr[P - 1, j, :] = t0 + P * j
        m["cnt_rows"] = _bf16(cr)
        in_maps.append(m)
    return in_maps


def _assemble(results):
    out = np.empty((B, T, D), np.float32)
    for c in range(8):
        b, half = divmod(c, 2)
        out[b, half * TH:(half + 1) * TH] = results[c]["out"]
    return out


def kernel(x, Wk, Wv, Wo, bo, g1, b1, Wf1, bf1, Wf2, bf2, g2, b2):
    lean = bool(
        not np.any(np.asarray(bo)) and not np.any(np.asarray(bf1))
        and not np.any(np.asarray(bf2)) and not np.any(np.asarray(b1))
        and not np.any(np.asarray(b2))
        and np.all(np.asarray(g1) == 1.0) and np.all(np.asarray(g2) == 1.0)
    )
    in_maps = _in_maps(x, Wv, Wo, bo, g1, b1, Wf1, bf1, Wf2, bf2, g2, b2)
    res = run_bass_kernel_spmd(_get_nc(lean), in_maps, list(range(8))).results
    return _assemble(res)


# revision 26
# speedup vs baseline: 1.2236x; 1.2236x over previous
"""TRN2 8-core SPMD kernel for nn_DecoderBlock_13443247636967.

Math note (validated to rel err ~1.5e-7 against the fp32 reference):
the reference uses SCALE = head_size**-5 = 2**-30 ~ 9.3e-10, so every
pre-softmax score satisfies |s| < 4e-8.  exp(s - max) is then 1.0 to
within one fp32 ulp and the reference softmax IS the uniform causal
average w_u = 1/(t+1) at fp32 precision.  Attention therefore reduces
to a causal prefix-mean of V, and the per-head structure fuses into a
single [D, D] value projection (Wk enters only through the vanishing
scores, so it cannot affect the output at fp32 resolution).

Sharding: core c = (batch b = c//2, half = c%2) owns 1024 sequence rows
of one batch.  The only cross-row coupling is the prefix sum.  Within a
core the 8 row-tiles form a short serial chain: the prefix matmul
C_ps = triu^T @ V already computes the full tile column-sum in its last
row, so row 127 of the (carry-added, unscaled) prefix PSUM *is* the
next tile's carry.  The carry is re-injected into PSUM with an all-ones
stationary matmul against a [P, D] tile whose partitions 1..127 are
zero.  The chain's root (sum over the other core's 1024 rows, pushed
through Wv) is a single [1, D] vector computed on the host during
input prep.  No collectives.

Precision: matmuls run in bf16 (keeps the PE's fast-weight-load path,
which f32r disables); residuals and LayerNorm stats stay fp32.
Measured end-to-end relative error vs the fp32 reference: ~1e-3-ish,
tolerance is 2e-2.
"""

import numpy as np
import ml_dtypes

import concourse.bass as bass
import concourse.mybir as mybir
import concourse.tile as tile
from concourse import bacc
from concourse.bass_utils import run_bass_kernel_spmd
from concourse.masks import make_identity

P = 128          # partitions / row-tile height
D = 1024         # model dim
TH = 1024        # sequence rows per core
NT = TH // P     # 8 row tiles
KC = D // P      # 8 contraction chunks
NF = 512         # matmul max moving free dim
NH = D // NF     # 2 column halves
B, T = 4, 2048
EPS = 1e-5
F32 = mybir.dt.float32
BF16 = mybir.dt.bfloat16


def _build(lean=True):
    # lean: biases known-zero and LN gains known-one (checked host-side;
    # the general variant is compiled on demand if that ever fails)
    nc = bacc.Bacc(
        "TRN2", target_bir_lowering=False, debug=False, num_devices=8
    )
    x = nc.dram_tensor("x_half", [TH, D], F32, kind="ExternalInput").ap()
    xT = nc.dram_tensor("xT_half", [NT, P, KC, P], BF16, kind="ExternalInput").ap()
    Wv = nc.dram_tensor("Wv", [D, D], BF16, kind="ExternalInput").ap()
    Wo = nc.dram_tensor("Wo", [D, D], BF16, kind="ExternalInput").ap()
    Wf1 = nc.dram_tensor("Wf1", [D, D], BF16, kind="ExternalInput").ap()
    Wf2 = nc.dram_tensor("Wf2", [D, D], BF16, kind="ExternalInput").ap()
    vecs = {
        name: nc.dram_tensor(name, [1, D], F32, kind="ExternalInput").ap()
        for name in ["bo", "bf1", "bf2", "g1", "b1", "g2", "b2"]
    }
    invcnt = nc.dram_tensor("invcnt", [P, NT], F32, kind="ExternalInput").ap()
    ut_in = nc.dram_tensor("ut_b", [P, P], BF16, kind="ExternalInput").ap()
    # carry0_t: zeros except row 127 = colsum(x_prev) @ Wv (host-computed)
    carry0 = nc.dram_tensor("carry0_t", [P, D], BF16, kind="ExternalInput").ap()
    # cnt_rows[127, j, :] = multiplier turning C_prev row 127 into carry_j
    cnt_in = nc.dram_tensor("cnt_rows", [P, NT, P], BF16, kind="ExternalInput").ap()
    out = nc.dram_tensor("out", [TH, D], F32, kind="ExternalOutput").ap()

    with tile.TileContext(nc) as tc:
        with tc.tile_pool(name="w", bufs=4) as wpool, \
             tc.tile_pool(name="xs", bufs=6) as xpool, \
             tc.tile_pool(name="bc", bufs=4) as bcpool, \
             tc.tile_pool(name="wkb", bufs=12) as wkb, \
             tc.tile_pool(name="wkf", bufs=8) as wkf, \
             tc.tile_pool(name="tp", bufs=6) as tppool, \
             tc.tile_pool(name="rows", bufs=1) as rows, \
             tc.tile_pool(name="stat", bufs=4) as statpool, \
             tc.tile_pool(name="pmm", bufs=6, space="PSUM") as pmm, \
             tc.tile_pool(name="ptp", bufs=2, space="PSUM") as ptp:

            # ---- constants ----
            ident = rows.tile([P, P], BF16)
            make_identity(nc, ident)
            ut_b = rows.tile([P, P], BF16)
            nc.sync.dma_start(out=ut_b, in_=ut_in)
            eps_t = rows.tile([P, 1], F32)
            nc.vector.memset(eps_t, EPS)
            icnt = rows.tile([P, NT], F32)
            nc.sync.dma_start(out=icnt, in_=invcnt)
            carry0_sb = rows.tile([P, D], BF16)
            nc.sync.dma_start(out=carry0_sb, in_=carry0)
            cnt_rows = rows.tile([P, NT, P], BF16)
            nc.sync.dma_start(out=cnt_rows, in_=cnt_in)

            def load_w(ap, name):
                w = wpool.tile([P, KC, D], BF16, tag="W", name=name)
                resh = ap.rearrange("(kc p) n -> p kc n", p=P)
                # one dma_start per kc chunk -> 8 rings in parallel
                for kc in range(KC):
                    nc.sync.dma_start(
                        out=w[:, kc:kc + 1, :], in_=resh[:, kc:kc + 1, :]
                    )
                return w

            def load_bc(name):
                t = bcpool.tile([P, D], F32, tag="bc", name=f"bc_{name}")
                nc.sync.dma_start(out=t, in_=vecs[name].to_broadcast([P, D]))
                return t

            def transpose_blocks(src, name):
                """src [P, D] bf16 natural -> [P, KC, P] bf16 blocks^T."""
                dst = tppool.tile([P, KC, P], BF16, tag="tp", name=name)
                for g in range(2):
                    tp_ps = ptp.tile([P, 4 * P], BF16, tag="ptp")
                    for k4 in range(4):
                        kc = g * 4 + k4
                        nc.tensor.transpose(
                            tp_ps[:, k4 * P:(k4 + 1) * P],
                            src[:, kc * P:(kc + 1) * P],
                            ident,
                        )
                    nc.vector.tensor_copy(
                        out=dst[:, g * 4:(g + 1) * 4, :],
                        in_=tp_ps.rearrange("p (k q) -> p k q", k=4),
                    )
                return dst

            def mm_group(lhsT_blocks, w_sb, n):
                """psum = sum_kc lhsT[:,kc,:].T @ w[:,kc,n-half]"""
                ps = pmm.tile([P, NF], F32, tag="mm")
                nsl = slice(n * NF, (n + 1) * NF)
                for kc in range(KC):
                    nc.tensor.matmul(
                        ps,
                        lhsT=lhsT_blocks[:, kc, :],
                        rhs=w_sb[:, kc, nsl],
                        start=(kc == 0),
                        stop=(kc == KC - 1),
                    )
                return ps

            def layernorm(src, dst, g_bc, b_bc, split=False):
                st = statpool.tile([P, NH, 6], F32, tag="st")
                for h in range(NH):
                    nc.vector.bn_stats(
                        out=st[:, h, :], in_=src[:, h * NF:(h + 1) * NF]
                    )
                mv = statpool.tile([P, 2], F32, tag="mv")
                nc.vector.bn_aggr(out=mv, in_=st)
                rstd = statpool.tile([P, 1], F32, tag="rs")
                nc.scalar.activation(
                    out=rstd,
                    in_=mv[:, 1:2],
                    func=mybir.ActivationFunctionType.Sqrt,
                    bias=eps_t,
                    scale=1.0,
                )
                nc.vector.reciprocal(out=rstd, in_=rstd)
                mb = statpool.tile([P, 1], F32, tag="mb")
                nc.vector.tensor_scalar(
                    out=mb, in0=mv[:, 0:1], scalar1=rstd, scalar2=-1.0,
                    op0=mybir.AluOpType.mult, op1=mybir.AluOpType.mult,
                )
                if split:
                    # tail latency: halves in parallel on DVE + ACT
                    nc.vector.tensor_scalar(
                        out=dst[:, 0:NF], in0=src[:, 0:NF],
                        scalar1=rstd, scalar2=mb,
                        op0=mybir.AluOpType.mult, op1=mybir.AluOpType.add,
                    )
                    nc.scalar.activation(
                        out=dst[:, NF:D], in_=src[:, NF:D],
                        func=mybir.ActivationFunctionType.Identity,
                        bias=mb, scale=rstd,
                    )
                else:
                    # normalize fully on ACT: keeps the DVE queue short
                    nc.scalar.activation(
                        out=dst, in_=src,
                        func=mybir.ActivationFunctionType.Identity,
                        bias=mb, scale=rstd,
                    )
                if not lean:
                    nc.vector.tensor_mul(out=dst, in0=dst, in1=g_bc)
                    nc.vector.tensor_add(out=dst, in0=dst, in1=b_bc)

            # ==== software-pipelined per-tile loop ====
            # attention of tile j is interleaved with the FFN of tile j-2
            # so every cross-engine wait on one stage is covered by
            # independent PE work from the other.
            state = {"C_prev": carry0_sb}
            xT_t = [None] * NT
            x_t = [None] * NT

            def fetch(j):
                if j >= NT or xT_t[j] is not None:
                    return
                xT_t[j] = tppool.tile([P, KC, P], BF16, tag="xT", name="xT")
                x_t[j] = xpool.tile([P, D], F32, tag="x", name="x1")
                for h in (0, 1):
                    ksl = slice(h * (KC // 2), (h + 1) * (KC // 2))
                    nc.sync.dma_start(out=xT_t[j][:, ksl, :], in_=xT[j][:, ksl, :])
                    nsl = slice(h * NF, (h + 1) * NF)
                    nc.sync.dma_start(
                        out=x_t[j][:, nsl], in_=x[j * P:(j + 1) * P, nsl]
                    )

            # fetch the first tiles BEFORE the weight DMAs so the first V
            # matmul isn't queued behind 8 MB of weight traffic
            fetch(0)
            fetch(1)
            Wv_sb = load_w(Wv, "Wv")
            Wo_sb = load_w(Wo, "Wo")
            Wf1_sb = load_w(Wf1, "Wf1")
            Wf2_sb = load_w(Wf2, "Wf2")
            bo_bc = None if lean else load_bc("bo")
            g1_bc = None if lean else load_bc("g1")
            b1_bc = None if lean else load_bc("b1")
            bf1_bc = None if lean else load_bc("bf1")
            bf2_bc = None if lean else load_bc("bf2")
            g2_bc = None if lean else load_bc("g2")
            b2_bc = None if lean else load_bc("b2")

            def copy_halves(dst, srcs, scalars=None, relu=False):
                """half 0 on DVE, half 1 on ACT (parallel engines)."""
                for n in range(NH):
                    nsl = slice(n * NF, (n + 1) * NF)
                    sc = scalars[n] if scalars is not None else 1.0
                    if n == 0:
                        if relu:
                            nc.vector.tensor_scalar_max(
                                out=dst[:, nsl], in0=srcs[n], scalar1=0.0
                            )
                        elif scalars is not None:
                            nc.vector.tensor_scalar_mul(
                                out=dst[:, nsl], in0=srcs[n], scalar1=sc
                            )
                        else:
                            nc.vector.tensor_copy(
                                out=dst[:, nsl], in_=srcs[n]
                            )
                    else:
                        fn = (mybir.ActivationFunctionType.Relu if relu
                              else mybir.ActivationFunctionType.Identity)
                        nc.scalar.activation(
                            out=dst[:, nsl], in_=srcs[n], func=fn,
                            scale=sc if scalars is not None else 1.0,
                        )

            def v_stage(j):
                """V = x @ Wv (+ bf16 staging copies)."""
                V_sb = wkb.tile([P, D], BF16, tag="wk", name="V")
                V_ps = [mm_group(xT_t[j], Wv_sb, n) for n in range(NH)]
                copy_halves(V_sb, V_ps)
                return V_sb

            def prefix_stage(j, V_sb):
                """C = (triu^T @ V + carry_j) * invcnt; carry_j comes from
                row 127 of the previous scaled C via a one-hot-row
                stationary operand holding the count."""
                C_b = wkb.tile([P, D], BF16, tag="wk", name="C")
                pss = []
                for n in range(NH):
                    nsl = slice(n * NF, (n + 1) * NF)
                    ps = pmm.tile([P, NF], F32, tag="mm")
                    nc.tensor.matmul(
                        ps, lhsT=ut_b, rhs=V_sb[:, nsl],
                        start=True, stop=False,
                    )
                    nc.tensor.matmul(
                        ps, lhsT=cnt_rows[:, j, :],
                        rhs=state["C_prev"][:, nsl],
                        start=False, stop=True,
                    )
                    pss.append(ps)
                copy_halves(C_b, pss, scalars=[icnt[:, j:j + 1]] * NH)
                state["C_prev"] = C_b
                return C_b

            def wo_stage(j, CT):
                """AO = C @ Wo ; r1 = AO + x."""
                r1 = wkf.tile([P, D], F32, tag="wk", name="r1")
                for n in range(NH):
                    nsl = slice(n * NF, (n + 1) * NF)
                    ps = mm_group(CT, Wo_sb, n)
                    if lean:
                        nc.vector.tensor_add(
                            out=r1[:, nsl], in0=ps, in1=x_t[j][:, nsl]
                        )
                    else:
                        nc.vector.tensor_add(
                            out=r1[:, nsl], in0=ps, in1=bo_bc[:, nsl]
                        )
                if not lean:
                    nc.vector.tensor_add(out=r1, in0=r1, in1=x_t[j])
                return r1

            def ln1_stage(r1):
                N1_b = wkb.tile([P, D], BF16, tag="wk", name="N1")
                layernorm(r1, N1_b, g1_bc, b1_bc)
                return N1_b

            def wf1_stage(N1T):
                """H = relu(N1 @ Wf1)."""
                H_b = wkb.tile([P, D], BF16, tag="wk", name="H")
                H_ps = [mm_group(N1T, Wf1_sb, n) for n in range(NH)]
                if lean:
                    copy_halves(H_b, H_ps, relu=True)
                else:
                    for n in range(NH):
                        nsl = slice(n * NF, (n + 1) * NF)
                        nc.vector.tensor_add(
                            out=H_b[:, nsl], in0=H_ps[n], in1=bf1_bc[:, nsl]
                        )
                    nc.vector.tensor_scalar_max(out=H_b, in0=H_b, scalar1=0.0)
                return H_b

            def wf2_stage(j, N1_b, HT, last=False):
                """z = H @ Wf2 + N1 + x ; out = LN2(z)."""
                z = wkf.tile([P, D], F32, tag="wk", name="z")
                for n in range(NH):
                    nsl = slice(n * NF, (n + 1) * NF)
                    ps = mm_group(HT, Wf2_sb, n)
                    if lean:
                        nc.vector.tensor_add(
                            out=z[:, nsl], in0=ps, in1=N1_b[:, nsl]
                        )
                    else:
                        nc.vector.tensor_add(
                            out=z[:, nsl], in0=ps, in1=bf2_bc[:, nsl]
                        )
                if not lean:
                    nc.vector.tensor_add(out=z, in0=z, in1=N1_b)
                if last:
                    # tail: DVE beats GpSimd's fixed overhead
                    nc.vector.tensor_add(out=z, in0=z, in1=x_t[j])
                else:
                    nc.gpsimd.tensor_add(out=z, in0=z, in1=x_t[j])
                o = wkf.tile([P, D], F32, tag="wk", name="o")
                layernorm(z, o, g2_bc, b2_bc, split=last)
                # 4-way split -> 4 rings; matters for the last tile's tail
                for q in range(4):
                    qsl = slice(q * (D // 4), (q + 1) * (D // 4))
                    nc.sync.dma_start(
                        out=out[j * P:(j + 1) * P, qsl], in_=o[:, qsl]
                    )

            # width-2 pipeline over tile pairs: the FFN of the previous
            # pair is threaded through the attention of the current pair
            # so every PSUM->SBUF handoff is covered by matmul work.
            prev = None  # (a, N1a, b, N1b)
            for i in range(NT // 2):
                a, b = 2 * i, 2 * i + 1
                Va = v_stage(a)
                Vb = v_stage(b)
                fetch(a + 2)
                fetch(b + 2)
                if prev:
                    pa, N1pa, pb, N1pb = prev
                    tpNa = transpose_blocks(N1pa, "N1T")
                    tpNb = transpose_blocks(N1pb, "N1T")
                Ca = prefix_stage(a, Va)
                Ha = wf1_stage(tpNa) if prev else None
                Cb = prefix_stage(b, Vb)
                Hb = wf1_stage(tpNb) if prev else None
                tca = transpose_blocks(Ca, "CT")
                tcb = transpose_blocks(Cb, "CT")
                if prev:
                    tpHa = transpose_blocks(Ha, "HT")
                    tpHb = transpose_blocks(Hb, "HT")
                r1a = wo_stage(a, tca)
                r1b = wo_stage(b, tcb)
                if prev:
                    wf2_stage(pa, N1pa, tpHa)
                    wf2_stage(pb, N1pb, tpHb)
                N1a = ln1_stage(r1a)
                N1b = ln1_stage(r1b)
                prev = (a, N1a, b, N1b)

            # epilogue: FFN of the last pair
            pa, N1pa, pb, N1pb = prev
            tpNa = transpose_blocks(N1pa, "N1T")
            tpNb = transpose_blocks(N1pb, "N1T")
            Ha = wf1_stage(tpNa)
            Hb = wf1_stage(tpNb)
            tpHa = transpose_blocks(Ha, "HT")
            tpHb = transpose_blocks(Hb, "HT")
            wf2_stage(pa, N1pa, tpHa)
            wf2_stage(pb, N1pb, tpHb, last=True)

    nc.compile()
    return nc


_CACHE = {}


def _get_nc(lean=True):
    key = "lean" if lean else "general"
    if key not in _CACHE:
        _CACHE[key] = _build(lean=lean)
    return _CACHE[key]


def _bf16(a):
    return np.ascontiguousarray(np.asarray(a, np.float32)).astype(
        ml_dtypes.bfloat16
    )


def _in_maps(x, Wv, Wo, bo, g1, b1, Wf1, bf1, Wf2, bf2, g2, b2):
    x = np.asarray(x, dtype=np.float32)
    Wv_all = np.ascontiguousarray(
        np.asarray(Wv, np.float32).transpose(1, 0, 2).reshape(D, D)
    )
    base = {
        "Wv": _bf16(Wv_all),
        "Wo": _bf16(Wo),
        "Wf1": _bf16(Wf1),
        "Wf2": _bf16(Wf2),
        "bo": np.asarray(bo, np.float32).reshape(1, D),
        "bf1": np.asarray(bf1, np.float32).reshape(1, D),
        "bf2": np.asarray(bf2, np.float32).reshape(1, D),
        "g1": np.asarray(g1, np.float32).reshape(1, D),
        "b1": np.asarray(b1, np.float32).reshape(1, D),
        "g2": np.asarray(g2, np.float32).reshape(1, D),
        "b2": np.asarray(b2, np.float32).reshape(1, D),
        "ut_b": _bf16(np.triu(np.ones((P, P), np.float32))),
    }
    in_maps = []
    for c in range(8):
        b, half = divmod(c, 2)
        t0 = half * TH
        icnt = 1.0 / (
            t0 + np.arange(P)[:, None] + P * np.arange(NT)[None, :] + 1.0
        )
        m = dict(base)
        xh = np.ascontiguousarray(x[b, t0:t0 + TH])
        m["x_half"] = xh
        # [NT, P, KC, P]: per row-tile j, partition p holds the KC
        # contraction blocks of x^T contiguously (2KB DMA lines)
        xt = xh.T.reshape(KC, P, NT, P).transpose(2, 1, 0, 3)
        m["xT_half"] = _bf16(np.ascontiguousarray(xt))
        m["invcnt"] = icnt.astype(np.float32)
        # prefix-sum root: column-sums of the other core's rows through Wv,
        # staged in row 127 of an otherwise-zero [P, D] tile
        c0 = np.zeros((P, D), np.float32)
        if half:
            c0[P - 1] = x[b, 0:TH].sum(axis=0) @ Wv_all
        m["carry0_t"] = _bf16(c0)
        # cnt_rows[127, 0] = 1 (consumes carry0 as-is); for j>=1 the
        # multiplier cnt = t0 + 128*j undoes invcnt on C_prev's row 127
        cr = np.zeros((P, NT, P), np.float32)
        cr[P - 1, 0, :] = 1.0
        for j in range(1, NT):
            cr[P - 1, j, :] = t0 + P * j
        m["cnt_rows"] = _bf16(cr)
        in_maps.append(m)
    return in_maps


def _assemble(results):
    out = np.empty((B, T, D), np.float32)
    for c in range(8):
        b, half = divmod(c, 2)
        out[b, half * TH:(half + 1) * TH] = results[c]["out"]
    return out


def kernel(x, Wk, Wv, Wo, bo, g1, b1, Wf1, bf1, Wf2, bf2, g2, b2):
    lean = bool(
        not np.any(np.asarray(bo)) and not np.any(np.asarray(bf1))
        and not np.any(np.asarray(bf2)) and not np.any(np.asarray(b1))
        and not np.any(np.asarray(b2))
        and np.all(np.asarray(g1) == 1.0) and np.all(np.asarray(g2) == 1.0)
    )
    in_maps = _in_maps(x, Wv, Wo, bo, g1, b1, Wf1, bf1, Wf2, bf2, g2, b2)
    res = run_bass_kernel_spmd(_get_nc(lean), in_maps, list(range(8))).results
    return _assemble(res)


# revision 38
# speedup vs baseline: 1.3334x; 1.0897x over previous
"""TRN2 8-core SPMD kernel for nn_DecoderBlock_13443247636967.

Math note (validated to rel err ~1.5e-7 against the fp32 reference):
the reference uses SCALE = head_size**-5 = 2**-30 ~ 9.3e-10, so every
pre-softmax score satisfies |s| < 4e-8.  exp(s - max) is then 1.0 to
within one fp32 ulp and the reference softmax IS the uniform causal
average w_u = 1/(t+1) at fp32 precision.  Attention therefore reduces
to a causal prefix-mean of V, and the per-head structure fuses into a
single [D, D] value projection (Wk enters only through the vanishing
scores, so it cannot affect the output at fp32 resolution).

Sharding: core c = (batch b = c//2, half = c%2) owns 1024 sequence rows
of one batch.  The only cross-row coupling is the prefix sum.  Within a
core the 8 row-tiles form a short serial chain: the prefix matmul
C_ps = triu^T @ V already computes the full tile column-sum in its last
row, so row 127 of the (carry-added, unscaled) prefix PSUM *is* the
next tile's carry.  The carry is re-injected into PSUM with an all-ones
stationary matmul against a [P, D] tile whose partitions 1..127 are
zero.  The chain's root (sum over the other core's 1024 rows, pushed
through Wv) is a single [1, D] vector computed on the host during
input prep.  No collectives.

Precision: matmuls run in bf16 (keeps the PE's fast-weight-load path,
which f32r disables); residuals and LayerNorm stats stay fp32.
Measured end-to-end relative error vs the fp32 reference: ~1e-3-ish,
tolerance is 2e-2.
"""

import numpy as np
import ml_dtypes

import concourse.bass as bass
import concourse.mybir as mybir
import concourse.tile as tile
from concourse import bacc
from concourse.bass_utils import run_bass_kernel_spmd
from concourse.masks import make_identity

P = 128          # partitions / row-tile height
D = 1024         # model dim
TH = 1024        # sequence rows per core
NT = TH // P     # 8 row tiles
KC = D // P      # 8 contraction chunks
NF = 512         # matmul max moving free dim
NH = D // NF     # 2 column halves
B, T = 4, 2048
EPS = 1e-5
F32 = mybir.dt.float32
BF16 = mybir.dt.bfloat16


def _build(lean=True):
    # lean: biases known-zero and LN gains known-one (checked host-side;
    # the general variant is compiled on demand if that ever fails)
    nc = bacc.Bacc(
        "TRN2", target_bir_lowering=False, debug=False, num_devices=8
    )
    x = nc.dram_tensor("x_half", [TH, D], BF16, kind="ExternalInput").ap()
    xT = nc.dram_tensor("xT_half", [NT, P, KC, P], BF16, kind="ExternalInput").ap()
    Wv = nc.dram_tensor("Wv", [D, D], BF16, kind="ExternalInput").ap()
    Wo = nc.dram_tensor("Wo", [D, D], BF16, kind="ExternalInput").ap()
    Wf1 = nc.dram_tensor("Wf1", [D, D], BF16, kind="ExternalInput").ap()
    Wf2 = nc.dram_tensor("Wf2", [D, D], BF16, kind="ExternalInput").ap()
    vecs = {
        name: nc.dram_tensor(name, [1, D], F32, kind="ExternalInput").ap()
        for name in ["bo", "bf1", "bf2", "g1", "b1", "g2", "b2"]
    }
    invcnt = nc.dram_tensor("invcnt", [P, NT], F32, kind="ExternalInput").ap()
    ut_in = nc.dram_tensor("ut_b", [P, P], BF16, kind="ExternalInput").ap()
    # carry0_t: zeros except row 127 = colsum(x_prev) @ Wv (host-computed)
    carry0 = nc.dram_tensor("carry0_t", [P, D], BF16, kind="ExternalInput").ap()
    # cnt_rows[127, j, :] = multiplier turning C_prev row 127 into carry_j
    cnt_in = nc.dram_tensor("cnt_rows", [P, NT, P], BF16, kind="ExternalInput").ap()
    out = nc.dram_tensor("out", [TH, D], F32, kind="ExternalOutput").ap()

    with tile.TileContext(nc) as tc:
        with tc.tile_pool(name="w", bufs=4) as wpool, \
             tc.tile_pool(name="xs", bufs=6) as xpool, \
             tc.tile_pool(name="bc", bufs=4) as bcpool, \
             tc.tile_pool(name="wkb", bufs=12) as wkb, \
             tc.tile_pool(name="wkf", bufs=6) as wkf, \
             tc.tile_pool(name="tp", bufs=6) as tppool, \
             tc.tile_pool(name="rows", bufs=1) as rows, \
             tc.tile_pool(name="stat", bufs=4) as statpool, \
             tc.tile_pool(name="pmm", bufs=2, space="PSUM") as pmm:

            # ---- constants ----
            ident = rows.tile([P, P], BF16)
            make_identity(nc, ident)
            ut_b = rows.tile([P, P], BF16)
            nc.sync.dma_start(out=ut_b, in_=ut_in)
            eps_t = rows.tile([P, 1], F32)
            nc.vector.memset(eps_t, EPS)
            icnt = rows.tile([P, NT], F32)
            nc.sync.dma_start(out=icnt, in_=invcnt)
            carry0_sb = rows.tile([P, D], BF16)
            nc.sync.dma_start(out=carry0_sb, in_=carry0)
            cnt_rows = rows.tile([P, NT, P], BF16)
            nc.sync.dma_start(out=cnt_rows, in_=cnt_in)

            def load_w(ap, name):
                w = wpool.tile([P, KC, D], BF16, tag="W", name=name)
                resh = ap.rearrange("(kc p) n -> p kc n", p=P)
                # 8 rings in parallel; half-0 columns land first so the
                # n=0 matmuls can start early (subtile deps)
                for h in range(NH):
                    nsl = slice(h * NF, (h + 1) * NF)
                    for g in range(4):
                        ksl = slice(g * 2, g * 2 + 2)
                        nc.sync.dma_start(
                            out=w[:, ksl, nsl], in_=resh[:, ksl, nsl]
                        )
                return w

            def load_bc(name):
                t = bcpool.tile([P, D], F32, tag="bc", name=f"bc_{name}")
                nc.sync.dma_start(out=t, in_=vecs[name].to_broadcast([P, D]))
                return t

            def transpose_blocks(src, name):
                """src [P, D] bf16 natural -> [P, KC, P] bf16 blocks^T."""
                dst = tppool.tile([P, KC, P], BF16, tag="tp", name=name)
                for g in range(2):
                    tp_ps = pmm.tile([P, 4 * P], BF16, tag="tp", bufs=2)
                    for k4 in range(4):
                        kc = g * 4 + k4
                        nc.tensor.transpose(
                            tp_ps[:, k4 * P:(k4 + 1) * P],
                            src[:, kc * P:(kc + 1) * P],
                            ident,
                        )
                    nc.vector.tensor_copy(
                        out=dst[:, g * 4:(g + 1) * 4, :],
                        in_=tp_ps.rearrange("p (k q) -> p k q", k=4),
                    )
                return dst

            def mm_group(lhsT_blocks, w_sb, n):
                """psum = sum_kc lhsT[:,kc,:].T @ w[:,kc,n-half]"""
                ps = pmm.tile([P, NF], F32, tag="mm")
                nsl = slice(n * NF, (n + 1) * NF)
                for kc in range(KC):
                    nc.tensor.matmul(
                        ps,
                        lhsT=lhsT_blocks[:, kc, :],
                        rhs=w_sb[:, kc, nsl],
                        start=(kc == 0),
                        stop=(kc == KC - 1),
                    )
                return ps

            def layernorm(srcs, dst, g_bc, b_bc, split=False):
                """srcs: per-half APs (SBUF or PSUM) of the LN input."""
                st = statpool.tile([P, NH, 6], F32, tag="st")
                for h in range(NH):
                    nc.vector.bn_stats(out=st[:, h, :], in_=srcs[h])
                mv = statpool.tile([P, 2], F32, tag="mv")
                nc.vector.bn_aggr(out=mv, in_=st)
                rstd = statpool.tile([P, 1], F32, tag="rs")
                nc.scalar.activation(
                    out=rstd,
                    in_=mv[:, 1:2],
                    func=mybir.ActivationFunctionType.Sqrt,
                    bias=eps_t,
                    scale=1.0,
                )
                nc.vector.reciprocal(out=rstd, in_=rstd)
                mb = statpool.tile([P, 1], F32, tag="mb")
                nc.vector.tensor_scalar(
                    out=mb, in0=mv[:, 0:1], scalar1=rstd, scalar2=-1.0,
                    op0=mybir.AluOpType.mult, op1=mybir.AluOpType.mult,
                )
                for h in range(NH):
                    nsl = slice(h * NF, (h + 1) * NF)
                    if split and h == 0:
                        # tail latency: halves in parallel on DVE + ACT
                        nc.vector.tensor_scalar(
                            out=dst[:, nsl], in0=srcs[h],
                            scalar1=rstd, scalar2=mb,
                            op0=mybir.AluOpType.mult, op1=mybir.AluOpType.add,
                        )
                    else:
                        # normalize on ACT: keeps the DVE queue short
                        nc.scalar.activation(
                            out=dst[:, nsl], in_=srcs[h],
                            func=mybir.ActivationFunctionType.Identity,
                            bias=mb, scale=rstd,
                        )
                if not lean:
                    nc.vector.tensor_mul(out=dst, in0=dst, in1=g_bc)
                    nc.vector.tensor_add(out=dst, in0=dst, in1=b_bc)

            # ==== software-pipelined per-tile loop ====
            # attention of tile j is interleaved with the FFN of tile j-2
            # so every cross-engine wait on one stage is covered by
            # independent PE work from the other.
            state = {"C_prev": carry0_sb}
            xT_t = [None] * NT
            x_t = [None] * NT

            def fetch(j):
                if j >= NT or xT_t[j] is not None:
                    return
                xT_t[j] = tppool.tile([P, KC, P], BF16, tag="xT", name="xT")
                x_t[j] = xpool.tile([P, D], BF16, tag="x", name="x1")
                for h in (0, 1):
                    ksl = slice(h * (KC // 2), (h + 1) * (KC // 2))
                    nc.sync.dma_start(out=xT_t[j][:, ksl, :], in_=xT[j][:, ksl, :])
                    nsl = slice(h * NF, (h + 1) * NF)
                    nc.sync.dma_start(
                        out=x_t[j][:, nsl], in_=x[j * P:(j + 1) * P, nsl]
                    )

            # fetch the first tiles BEFORE the weight DMAs so the first V
            # matmul isn't queued behind 8 MB of weight traffic
            fetch(0)
            fetch(1)
            Wv_sb = load_w(Wv, "Wv")
            Wo_sb = load_w(Wo, "Wo")
            Wf1_sb = load_w(Wf1, "Wf1")
            Wf2_sb = load_w(Wf2, "Wf2")
            bo_bc = None if lean else load_bc("bo")
            g1_bc = None if lean else load_bc("g1")
            b1_bc = None if lean else load_bc("b1")
            bf1_bc = None if lean else load_bc("bf1")
            bf2_bc = None if lean else load_bc("bf2")
            g2_bc = None if lean else load_bc("g2")
            b2_bc = None if lean else load_bc("b2")

            def copy_halves(dst, srcs, scalars=None, relu=False):
                """half 0 on DVE, half 1 on ACT (parallel engines)."""
                for n in range(NH):
                    nsl = slice(n * NF, (n + 1) * NF)
                    sc = scalars[n] if scalars is not None else 1.0
                    if n == 0:
                        if relu:
                            nc.vector.tensor_scalar_max(
                                out=dst[:, nsl], in0=srcs[n], scalar1=0.0
                            )
                        elif scalars is not None:
                            nc.vector.tensor_scalar_mul(
                                out=dst[:, nsl], in0=srcs[n], scalar1=sc
                            )
                        else:
                            nc.vector.tensor_copy(
                                out=dst[:, nsl], in_=srcs[n]
                            )
                    else:
                        fn = (mybir.ActivationFunctionType.Relu if relu
                              else mybir.ActivationFunctionType.Identity)
                        nc.scalar.activation(
                            out=dst[:, nsl], in_=srcs[n], func=fn,
                            scale=sc if scalars is not None else 1.0,
                        )

            def v_stage(j):
                """V = x @ Wv (+ bf16 staging copies)."""
                V_sb = wkb.tile([P, D], BF16, tag="wk", name="V")
                V_ps = [mm_group(xT_t[j], Wv_sb, n) for n in range(NH)]
                copy_halves(V_sb, V_ps)
                return V_sb

            def prefix_stage(j, V_sb):
                """C = (triu^T @ V + carry_j) * invcnt; carry_j comes from
                row 127 of the previous scaled C via a one-hot-row
                stationary operand holding the count."""
                C_b = wkb.tile([P, D], BF16, tag="wk", name="C")
                pss = []
                for n in range(NH):
                    nsl = slice(n * NF, (n + 1) * NF)
                    ps = pmm.tile([P, NF], F32, tag="mm")
                    nc.tensor.matmul(
                        ps, lhsT=ut_b, rhs=V_sb[:, nsl],
                        start=True, stop=False,
                    )
                    nc.tensor.matmul(
                        ps, lhsT=cnt_rows[:, j, :],
                        rhs=state["C_prev"][:, nsl],
                        start=False, stop=True,
                    )
                    pss.append(ps)
                copy_halves(C_b, pss, scalars=[icnt[:, j:j + 1]] * NH)
                state["C_prev"] = C_b
                return C_b

            def wo_stage(j, CT):
                """r1 = C @ Wo + x, accumulated fully in PSUM: the x
                residual is injected with an identity-matmul so no
                elementwise engine sits between the PE and LN1."""
                pss = []
                for n in range(NH):
                    nsl = slice(n * NF, (n + 1) * NF)
                    ps = pmm.tile([P, NF], F32, tag="r1", bufs=4, name="r1")
                    for kc in range(KC):
                        nc.tensor.matmul(
                            ps, lhsT=CT[:, kc, :], rhs=Wo_sb[:, kc, nsl],
                            start=(kc == 0), stop=False,
                        )
                    nc.tensor.matmul(
                        ps, lhsT=ident, rhs=x_t[j][:, nsl],
                        start=False, stop=True,
                    )
                    pss.append(ps)
                if not lean:
                    for n in range(NH):
                        nsl = slice(n * NF, (n + 1) * NF)
                        nc.vector.tensor_add(
                            out=pss[n], in0=pss[n], in1=bo_bc[:, nsl]
                        )
                return pss

            def ln1_stage(r1_ps):
                N1_b = wkb.tile([P, D], BF16, tag="wk", name="N1")
                layernorm(r1_ps, N1_b, g1_bc, b1_bc)
                return N1_b

            def wf1_stage(N1T):
                """H = relu(N1 @ Wf1)."""
                H_b = wkb.tile([P, D], BF16, tag="wk", name="H")
                H_ps = [mm_group(N1T, Wf1_sb, n) for n in range(NH)]
                if lean:
                    copy_halves(H_b, H_ps, relu=True)
                else:
                    for n in range(NH):
                        nsl = slice(n * NF, (n + 1) * NF)
                        nc.vector.tensor_add(
                            out=H_b[:, nsl], in0=H_ps[n], in1=bf1_bc[:, nsl]
                        )
                    nc.vector.tensor_scalar_max(out=H_b, in0=H_b, scalar1=0.0)
                return H_b

            def wf2_stage(j, N1_b, HT, last=False):
                """z = H @ Wf2 + N1 + x ; out = LN2(z)."""
                z = wkf.tile([P, D], F32, tag="wk", name="z")
                for n in range(NH):
                    nsl = slice(n * NF, (n + 1) * NF)
                    ps = mm_group(HT, Wf2_sb, n)
                    if lean:
                        nc.vector.tensor_add(
                            out=z[:, nsl], in0=ps, in1=N1_b[:, nsl]
                        )
                    else:
                        nc.vector.tensor_add(
                            out=z[:, nsl], in0=ps, in1=bf2_bc[:, nsl]
                        )
                if not lean:
                    nc.vector.tensor_add(out=z, in0=z, in1=N1_b)
                if last:
                    # tail: DVE beats GpSimd's fixed overhead
                    nc.vector.tensor_add(out=z, in0=z, in1=x_t[j])
                else:
                    nc.gpsimd.tensor_add(out=z, in0=z, in1=x_t[j])
                o = wkf.tile([P, D], F32, tag="wk", name="o")
                layernorm(
                    [z[:, 0:NF], z[:, NF:D]], o, g2_bc, b2_bc, split=last
                )
                # 4-way split -> 4 rings; matters for the last tile's tail
                for q in range(4):
                    qsl = slice(q * (D // 4), (q + 1) * (D // 4))
                    nc.sync.dma_start(
                        out=out[j * P:(j + 1) * P, qsl], in_=o[:, qsl]
                    )

            # width-2 pipeline over tile pairs: the FFN of the previous
            # pair is threaded through the attention of the current pair
            # so every PSUM->SBUF handoff is covered by matmul work.
            prev = None  # (a, N1a, b, N1b)
            for i in range(NT // 2):
                a, b = 2 * i, 2 * i + 1
                Va = v_stage(a)
                Vb = v_stage(b)
                fetch(a + 2)
                fetch(b + 2)
                if prev:
                    pa, N1pa, pb, N1pb = prev
                    tpNa = transpose_blocks(N1pa, "N1T")
                    tpNb = transpose_blocks(N1pb, "N1T")
                Ca = prefix_stage(a, Va)
                Ha = wf1_stage(tpNa) if prev else None
                Cb = prefix_stage(b, Vb)
                Hb = wf1_stage(tpNb) if prev else None
                tca = transpose_blocks(Ca, "CT")
                tcb = transpose_blocks(Cb, "CT")
                if prev:
                    tpHa = transpose_blocks(Ha, "HT")
                    tpHb = transpose_blocks(Hb, "HT")
                r1a = wo_stage(a, tca)
                r1b = wo_stage(b, tcb)
                N1a = ln1_stage(r1a)
                N1b = ln1_stage(r1b)
                if prev:
                    wf2_stage(pa, N1pa, tpHa)
                    wf2_stage(pb, N1pb, tpHb)
                prev = (a, N1a, b, N1b)

            # epilogue: FFN of the last pair
            pa, N1pa, pb, N1pb = prev
            tpNa = transpose_blocks(N1pa, "N1T")
            tpNb = transpose_blocks(N1pb, "N1T")
            Ha = wf1_stage(tpNa)
            Hb = wf1_stage(tpNb)
            tpHa = transpose_blocks(Ha, "HT")
            tpHb = transpose_blocks(Hb, "HT")
            wf2_stage(pa, N1pa, tpHa)
            wf2_stage(pb, N1pb, tpHb, last=True)

    nc.compile()
    return nc


_CACHE = {}


def _get_nc(lean=True):
    key = "lean" if lean else "general"
    if key not in _CACHE:
        _CACHE[key] = _build(lean=lean)
    return _CACHE[key]


def _bf16(a):
    return np.ascontiguousarray(np.asarray(a, np.float32)).astype(
        ml_dtypes.bfloat16
    )


def _in_maps(x, Wv, Wo, bo, g1, b1, Wf1, bf1, Wf2, bf2, g2, b2):
    x = np.asarray(x, dtype=np.float32)
    Wv_all = np.ascontiguousarray(
        np.asarray(Wv, np.float32).transpose(1, 0, 2).reshape(D, D)
    )
    base = {
        "Wv": _bf16(Wv_all),
        "Wo": _bf16(Wo),
        "Wf1": _bf16(Wf1),
        "Wf2": _bf16(Wf2),
        "bo": np.asarray(bo, np.float32).reshape(1, D),
        "bf1": np.asarray(bf1, np.float32).reshape(1, D),
        "bf2": np.asarray(bf2, np.float32).reshape(1, D),
        "g1": np.asarray(g1, np.float32).reshape(1, D),
        "b1": np.asarray(b1, np.float32).reshape(1, D),
        "g2": np.asarray(g2, np.float32).reshape(1, D),
        "b2": np.asarray(b2, np.float32).reshape(1, D),
        "ut_b": _bf16(np.triu(np.ones((P, P), np.float32))),
    }
    in_maps = []
    for c in range(8):
        b, half = divmod(c, 2)
        t0 = half * TH
        icnt = 1.0 / (
            t0 + np.arange(P)[:, None] + P * np.arange(NT)[None, :] + 1.0
        )
        m = dict(base)
        xh = np.ascontiguousarray(x[b, t0:t0 + TH])
        m["x_half"] = _bf16(xh)
        # [NT, P, KC, P]: per row-tile j, partition p holds the KC
        # contraction blocks of x^T contiguously (2KB DMA lines)
        xt = xh.T.reshape(KC, P, NT, P).transpose(2, 1, 0, 3)
        m["xT_half"] = _bf16(np.ascontiguousarray(xt))
        m["invcnt"] = icnt.astype(np.float32)
        # prefix-sum root: column-sums of the other core's rows through Wv,
        # staged in row 127 of an otherwise-zero [P, D] tile
        c0 = np.zeros((P, D), np.float32)
        if half:
            c0[P - 1] = x[b, 0:TH].sum(axis=0) @ Wv_all
        m["carry0_t"] = _bf16(c0)
        # cnt_rows[127, 0] = 1 (consumes carry0 as-is); for j>=1 the
        # multiplier cnt = t0 + 128*j undoes invcnt on C_prev's row 127
        cr = np.zeros((P, NT, P), np.float32)
        cr[P - 1, 0, :] = 1.0
        for j in range(1, NT):
            cr[P - 1, j, :] = t0 + P * j
        m["cnt_rows"] = _bf16(cr)
        in_maps.append(m)
    return in_maps


def _assemble(results):
    out = np.empty((B, T, D), np.float32)
    for c in range(8):
        b, half = divmod(c, 2)
        out[b, half * TH:(half + 1) * TH] = results[c]["out"]
    return out


def kernel(x, Wk, Wv, Wo, bo, g1, b1, Wf1, bf1, Wf2, bf2, g2, b2):
    lean = bool(
        not np.any(np.asarray(bo)) and not np.any(np.asarray(bf1))
        and not np.any(np.asarray(bf2)) and not np.any(np.asarray(b1))
        and not np.any(np.asarray(b2))
        and np.all(np.asarray(g1) == 1.0) and np.all(np.asarray(g2) == 1.0)
    )
    in_maps = _in_maps(x, Wv, Wo, bo, g1, b1, Wf1, bf1, Wf2, bf2, g2, b2)
    res = run_bass_kernel_spmd(_get_nc(lean), in_maps, list(range(8))).results
    return _assemble(res)
